# revision 52
# baseline (speedup 1.0000x reference)
"""Conv2d(128->256, 3x3, pad 1, stride 1) on 32x56x56 fp32, for 8 trn2 cores.

Strategy: data-parallel over batch N=32 -> 4 images/core. Per core an
implicit-GEMM conv: C_in=128 is the partition (contraction) dim; for each
(kh, kw) tap a [128ci x 128co] weight tile multiplies a shifted window of the
column-padded input image held in SBUF, accumulating into PSUM over the 9 taps.
Output rows are processed in chunks of 8 (free dim 8*56=448 <= 512 PSUM bank).
Matmuls run in float16 (fp16 keeps ~2.6e-4 rel err) with fp32 PSUM accumulate.

Layout details (all tuned from perfetto traces):
- Two SBUF copies of each input slice: copy A with the row interior at column
  1 (serves kw=0 and kw=2 taps) and copy B at column 2 (serves kw=1). This
  keeps every matmul rhs at an even fp16 element offset; odd offsets cost ~18
  extra PE cycles per matmul (SBUF word-split reads).
- No zero pad ROWS: boundary chunks instead shrink the kh taps that would
  read them (448 -> 392 free, PSUM sub-range), saving ~1.1us of PE work.
  Copy A keeps zero pad COLUMNS (elems 0 and 57 of each 58-elem row).
- PE warmup: dummy matmuls opened ASAP (wu memset on the gpsimd queue, which
  starts ~1us before vector) keep the PE busy while head DMAs land, so the
  HAM clock gate (opens ~6us after first PE activity, half clock until then)
  is already open when the real stream runs.
- Head DMAs: first x slice is only 9 rows (enough for chunk 0) and the tap-
  first weight block is a separate small DMA, so the first real matmul's
  dependencies land as early as possible.
- Outputs: full-half bulk DMAs for images 0..2 (12.5KB/partition
  descriptors), but CHUNK-wise for the whole last image so the final chunk's
  writeout is not queued behind a 1.6MB bulk transfer in the DMA FIFOs.
"""
import numpy as np
from contextlib import ExitStack

N_FULL, C_IN, H, W = 32, 128, 56, 56
C_OUT, KS = 256, 3
N_CORES = 8
N_PER = N_FULL // N_CORES          # 4 images per core
PIX = H * W                         # 3136
ROWS = 8                            # output rows per psum chunk
RC = H // ROWS                      # 7 chunks
NF = ROWS * W                       # 448 free elems per matmul
NARROW = (ROWS - 1) * W             # 392 free elems for boundary taps
SW = 58                             # padded row stride (56 + 2 border cols)

T_ROWS = 33                         # top tile: x rows 0..32 (chunks 0-3)
B_ROWS = 25                         # bottom tile: x rows 31..55 (chunks 4-6)
B0 = 31                             # first x row held in the bottom tile
XT_A = 9                            # first sub-DMA: x rows 0..8 (chunk 0)
XT_B = 12                           # second: x rows 9..20
XT_C = T_ROWS - XT_A - XT_B         # third: x rows 21..32

# tap order per chunk kind: (kh, kw) tuples. start=first, stop=last.
# narrow taps (boundary rows) must not be first or last.
TAPS_MID = [(0, 0), (0, 1), (0, 2), (1, 0), (1, 1), (1, 2), (2, 0), (2, 1), (2, 2)]
# top chunk leads with the kw=1 taps: for image 0 they run straight off the
# raw (unpadded) first DMA slice -- the center column needs no padding and
# raw rows are contiguous -- so real work starts before any pad copy lands
TAPS_TOP = [(1, 1), (2, 1), (0, 1), (1, 0), (1, 2), (0, 0), (0, 2), (2, 0), (2, 2)]
TAPS_BOT = [(0, 0), (0, 1), (0, 2), (2, 0), (2, 1), (2, 2), (1, 0), (1, 1), (1, 2)]

_CACHE = {}


def _build():
    import concourse.tile as tile
    from concourse import mybir, bacc

    f32 = mybir.dt.float32
    f16 = mybir.dt.float16

    nc = bacc.Bacc("TRN2", target_bir_lowering=False, debug=False)
    x_d = nc.dram_tensor("x", [N_PER, C_IN, PIX], f16, kind="ExternalInput").ap()
    # host-pretransposed: [ci, half, k, co_half] (half-major, contiguous per
    # half); within each half tap k=3 (the first tap chunk 0 runs) is stored
    # first: host order TAPS_TOP.
    w_d = nc.dram_tensor("w", [C_IN, 2, KS * KS, 128], f16, kind="ExternalInput").ap()
    b_d = nc.dram_tensor("b", [C_OUT], f32, kind="ExternalInput").ap()
    # fp16 output (upcast on host): halves output DMA bytes; ~5e-4 rel err
    y_d = nc.dram_tensor("y", [N_PER, C_OUT, PIX], f16, kind="ExternalOutput").ap()

    # host-order index of tap (kh, kw) inside a weight half (TAPS_TOP order)
    widx = {t: i for i, t in enumerate(TAPS_TOP)}

    with tile.TileContext(nc) as tc:
        with ExitStack() as ctx:
            wp = ctx.enter_context(tc.tile_pool(name="wp", bufs=1))
            xrawta = ctx.enter_context(tc.tile_pool(name="xrawta", bufs=1))
            xrawtb = ctx.enter_context(tc.tile_pool(name="xrawtb", bufs=1))
            xrawtc = ctx.enter_context(tc.tile_pool(name="xrawtc", bufs=1))
            xrawb = ctx.enter_context(tc.tile_pool(name="xrawb", bufs=1))
            # images 1-3 land whole, prefetched at kernel start with no deps:
            # their descriptors sit behind image-0's in the ring FIFOs, and all
            # input traffic completes before the first bulk output DMA can
            # starve it (big output descriptors occupy a queue ~0.5us each).
            xrawf = ctx.enter_context(tc.tile_pool(name="xrawf", bufs=N_PER - 1))
            xpadt = ctx.enter_context(tc.tile_pool(name="xpadt", bufs=4))
            xpadb = ctx.enter_context(tc.tile_pool(name="xpadb", bufs=4))
            pp = ctx.enter_context(tc.tile_pool(name="pp", bufs=6, space="PSUM"))
            op = ctx.enter_context(tc.tile_pool(name="op", bufs=2))

            # PE warmup: the clock ramps to full speed only after ~3-6us of
            # CONTINUOUS PE execution, and re-gates after ~1us idle. Keep the
            # PE busy with dummies from as early as possible until real work
            # is ready (~8.5us). The first dummies read the framework's
            # const-AP (materialized during init, no runtime memset dep) so
            # they start right at the post-init barrier on every core; then
            # wu-based dummies, shrinking toward the end for a fine handoff.
            c0 = nc.const_aps.tensor(0.0, [128, 1], f32)
            wups = pp.tile([128, NF], f32, tag="ps")
            for _ in range(10):
                nc.tensor.matmul(wups[0:1, 0:1], c0, c0, start=True, stop=True)
            wu = wp.tile([128, 224], f16)
            nc.gpsimd.memset(wu[:], 0.0)
            # overprovision the runway: a core whose inputs land late pays
            # ~1.5us for a PE idle (gap + clock re-gate), while an excess
            # dummy costs only ~45ns of displaced half-clock work
            for _ in range(22):
                nc.tensor.matmul(wups[:, 0:112], wu[:, 0:128], wu[:, 0:112], start=True, stop=True)

            # Weight half 0 in two pieces on the ACT ring: chunk 0's first six
            # taps' weights (192KB) gate the first matmuls; the rest follows.
            w_r = wp.tile([C_IN, 2 * KS * KS * 128], f16)
            w_r4 = w_r[:].rearrange("p (h k co) -> p h k co", h=2, k=KS * KS)
            nc.scalar.dma_start(
                w_r4[:, 0, 0:3], w_d[:, 0, 0:3].rearrange("ci k co -> ci (k co)")
            )
            nc.scalar.dma_start(
                w_r4[:, 0, 3:9], w_d[:, 0, 3:9].rearrange("ci k co -> ci (k co)")
            )

            bias_sb = wp.tile([128, 2], f32)

            for n in range(N_PER):
                if n == 0:
                    # image 0: top interior in three slices so chunk 0
                    # unblocks as early as possible
                    xrta = xrawta.tile([C_IN, XT_A * W], f16)
                    nc.sync.dma_start(xrta[:], x_d[n, :, 0 : XT_A * W])
                    xrtb = xrawtb.tile([C_IN, XT_B * W], f16)
                    nc.sync.dma_start(xrtb[:], x_d[n, :, XT_A * W : (XT_A + XT_B) * W])
                    xrtc = xrawtc.tile([C_IN, XT_C * W], f16)
                    nc.sync.dma_start(xrtc[:], x_d[n, :, (XT_A + XT_B) * W : T_ROWS * W])
                    xrb = xrawb.tile([C_IN, B_ROWS * W], f16)
                    nc.sync.dma_start(xrb[:], x_d[n, :, B0 * W : (B0 + B_ROWS) * W])
                    top_srcs = [(xrta[:], 0, XT_A), (xrtb[:], XT_A, XT_A + XT_B),
                                (xrtc[:], XT_A + XT_B, T_ROWS)]
                    bot_src = xrb[:]
                    # weight half 1 + bias queue behind the critical pieces
                    nc.scalar.dma_start(
                        w_r4[:, 1], w_d[:, 1].rearrange("ci k co -> ci (k co)")
                    )
                    nc.scalar.dma_start(bias_sb[:], b_d.rearrange("(h p) -> p h", h=2))
                else:
                    # images 1-3: one whole-image DMA, prefetched (no deps)
                    xrf = xrawf.tile([C_IN, PIX], f16)
                    nc.sync.dma_start(xrf[:], x_d[n])
                    top_srcs = [(xrf[:, 0 : T_ROWS * W], 0, T_ROWS)]
                    bot_src = xrf[:, B0 * W : (B0 + B_ROWS) * W]

                # copy A: interior at col 1 (kw=0,2 taps; even rhs offsets
                # 0/2); zero border cols 0 and 57.  copy B: interior at col 2
                # (kw=1 taps; even rhs offset 2), no borders needed.
                xptA = xpadt.tile([C_IN, T_ROWS * SW], f16)
                xptA3 = xptA[:].rearrange("p (a b) -> p a b", a=T_ROWS)
                xptB = xpadt.tile([C_IN, T_ROWS * SW], f16)
                xptB3 = xptB[:].rearrange("p (a b) -> p a b", a=T_ROWS)
                nc.vector.memset(xptA3[:, :, 0:1], 0.0)
                nc.vector.memset(xptA3[:, :, SW - 1 : SW], 0.0)
                # all copies on the DVE (Pool runs them ~6.5x slower); the
                # bias-adds live on the ACT engine instead, so a long copy
                # can never head-of-line-block an ADD (which would back up
                # PSUM and stall the PE at image boundaries).
                # interleave A/B per slice: chunk 0 needs both A and B of the
                # first slice ASAP
                for (src, a0, a1) in top_srcs:
                    src3 = src.rearrange("p (a b) -> p a b", a=a1 - a0)
                    nc.vector.tensor_copy(xptA3[:, a0:a1, 1 : 1 + W], src3)
                    if n == 0:
                        # images 1-3 run kw=1 taps off the raw tile directly;
                        # only image 0 (split raw pieces) needs the B copy
                        nc.vector.tensor_copy(xptB3[:, a0:a1, 2 : 2 + W], src3)

                xpbA = xpadb.tile([C_IN, B_ROWS * SW], f16)
                xpbA3 = xpbA[:].rearrange("p (a b) -> p a b", a=B_ROWS)
                xpbB = xpadb.tile([C_IN, B_ROWS * SW], f16)
                xpbB3 = xpbB[:].rearrange("p (a b) -> p a b", a=B_ROWS)
                nc.vector.memset(xpbA3[:, :, 0:1], 0.0)
                nc.vector.memset(xpbA3[:, :, SW - 1 : SW], 0.0)
                xrb3 = bot_src.rearrange("p (a b) -> p a b", a=B_ROWS)
                nc.vector.tensor_copy(xpbA3[:, :, 1 : 1 + W], xrb3)
                if n == 0:
                    nc.vector.tensor_copy(xpbB3[:, :, 2 : 2 + W], xrb3)

                out_sb = op.tile([128, 2 * PIX], f16)
                last_img = n == N_PER - 1
                for half in range(2):
                    # the very last half ends with 6-row + 2-row chunks so the
                    # post-final-matmul ADD+DMA tail is tiny
                    if last_img and half == 1:
                        chunks = [(i * ROWS, ROWS) for i in range(6)] + [(48, 6), (54, 2)]
                    else:
                        chunks = [(i * ROWS, ROWS) for i in range(RC)]
                    for ci, (r0c, nr) in enumerate(chunks):
                        nfc = nr * W
                        ps = pp.tile([128, NF], f32)
                        top_chunk = r0c == 0
                        bot_chunk = r0c + nr == H
                        taps = TAPS_TOP if top_chunk else (TAPS_BOT if bot_chunk else TAPS_MID)
                        for i, (kh, kw) in enumerate(taps):
                            lhsT = w_r4[:, half, widx[(kh, kw)], :]
                            # output rows r0c..r0c+nr-1 read x rows r0c+kh-1..
                            r0 = r0c + kh - 1
                            narrow_top = top_chunk and kh == 0     # skip out row 0
                            narrow_bot = bot_chunk and kh == 2     # skip out row 55
                            if r0c < 32:
                                A3, B3, base = xptA3, xptB3, 0
                            else:
                                A3, B3, base = xpbA3, xpbB3, B0
                            src3 = B3 if kw == 1 else A3
                            coff = 2 if kw == 1 else kw
                            if n == 0 and top_chunk and kw == 1:
                                # image 0's first chunk: kw=1 taps straight
                                # off the raw DMA slice (no pad copy dep)
                                src3 = xrta[:].rearrange("p (a b) -> p a b", a=XT_A)
                                coff = 0
                            if kw == 1 and n > 0:
                                # kw=1 needs no column padding: read the raw
                                # whole-image tile as ONE contiguous free run.
                                # A row-structured rhs AP costs ~1 PE cycle
                                # per row boundary; the flat run costs none.
                                nrows = nr - 1 if (narrow_top or narrow_bot) else nr
                                rr0 = 0 if narrow_top else r0
                                rhs = xrf[:, rr0 * W : (rr0 + nrows) * W]
                                if narrow_top:
                                    dst = ps[:, W:nfc]
                                elif narrow_bot:
                                    dst = ps[:, 0 : nfc - W]
                                else:
                                    dst = ps[:, 0:nfc]
                            elif narrow_top:
                                rhs = src3[:, 0 : nr - 1, coff : coff + W]
                                dst = ps[:, W:nfc]
                            elif narrow_bot:
                                lr = r0 - base
                                rhs = src3[:, lr : lr + nr - 1, coff : coff + W]
                                dst = ps[:, 0 : nfc - W]
                            else:
                                lr = r0 - base
                                rhs = src3[:, lr : lr + nr, coff : coff + W]
                                dst = ps[:, 0:nfc]
                            nc.tensor.matmul(
                                dst, lhsT, rhs,
                                start=(i == 0), stop=(i == KS * KS - 1),
                            )
                        # psum -> sbuf with per-channel bias add (f32 -> f16)
                        # on the ACT engine: keeps the DVE free for pad copies
                        lo = half * PIX + r0c * W
                        nc.scalar.activation(
                            out_sb[:, lo : lo + nfc],
                            ps[:, 0:nfc],
                            mybir.ActivationFunctionType.Identity,
                            bias=bias_sb[:, half : half + 1],
                        )
                        if last_img:
                            # last image: stream output out as produced so
                            # nothing bulky queues ahead of the tail. Pair up
                            # 8-row chunks (fewer DMAs -> fewer semaphores to
                            # clear in teardown); alternate rings so
                            # consecutive writes drain in parallel.
                            paired = (
                                nr == ROWS and ci % 2 == 0
                                and ci + 1 < len(chunks) and chunks[ci + 1][1] == ROWS
                            )
                            if paired:
                                pass  # written together with the next chunk
                            else:
                                if nr == 6:
                                    continue  # written merged with the final mini chunk
                                w0 = (r0c - ROWS) * W if (nr == ROWS and ci % 2 == 1) else r0c * W
                                if nr == 2:
                                    # final mini chunk: one merged 8-row write
                                    # (rows 48-55; bigger descriptors, single
                                    # issue op) on the scalar queue right
                                    # behind its own bias-add -- the in-order
                                    # queue guarantees the 6-row chunk's add
                                    # already ran, and no cross-engine
                                    # semaphore hop on the tail-critical DMA
                                    w0 = (r0c - 6) * W
                                    eng = nc.scalar
                                else:
                                    eng = nc.sync if (half == 0) == (ci % 2 == 0) else nc.gpsimd
                                eng.dma_start(
                                    y_d[n, half * 128 : (half + 1) * 128, w0 : r0c * W + nfc],
                                    out_sb[:, half * PIX + w0 : lo + nfc],
                                )
                    if not last_img:
                        eng = nc.scalar if half == 0 else nc.gpsimd
                        eng.dma_start(
                            y_d[n, half * 128 : (half + 1) * 128, :],
                            out_sb[:, half * PIX : (half + 1) * PIX],
                        )
    nc.compile()
    return nc


def _get_nc():
    if "nc" not in _CACHE:
        _CACHE["nc"] = _build()
    return _CACHE["nc"]


def _prep_inputs(x, weight, bias):
    # fp16 on host: halves input DMA bytes and drops the on-device casts
    x = np.ascontiguousarray(
        np.asarray(x, dtype=np.float32).astype(np.float16).reshape(N_FULL, C_IN, PIX)
    )
    # [co, ci, kh, kw] -> [ci, half, k, co_half], half-major; taps within a
    # half stored in TAPS_TOP order so chunk-0's first weights DMA first.
    w4 = (
        np.transpose(np.asarray(weight, dtype=np.float32), (1, 2, 3, 0))
        .reshape(C_IN, KS * KS, 2, 128)
        .transpose(0, 2, 1, 3)
    )  # [ci, half, k(row-major), co]
    perm = [kh * KS + kw for (kh, kw) in TAPS_TOP]
    w_t = np.ascontiguousarray(w4[:, :, perm, :].astype(np.float16))
    b = np.ascontiguousarray(bias, dtype=np.float32)
    return x, w_t, b


def kernel(x, weight, bias):
    from concourse.bass_utils import run_bass_kernel_spmd

    x, w_t, b = _prep_inputs(x, weight, bias)
    nc = _get_nc()
    in_maps = [
        {"x": x[i * N_PER : (i + 1) * N_PER], "w": w_t, "b": b}
        for i in range(N_CORES)
    ]
    res = run_bass_kernel_spmd(nc, in_maps, list(range(N_CORES)))
    y = np.concatenate(
        [
            res.results[i]["y"].reshape(N_PER, C_OUT, H, W).astype(np.float32)
            for i in range(N_CORES)
        ],
        axis=0,
    )
    return y


# revision 53
# speedup vs baseline: 1.0131x; 1.0131x over previous
"""Conv2d(128->256, 3x3, pad 1, stride 1) on 32x56x56 fp32, for 8 trn2 cores.

Strategy: data-parallel over batch N=32 -> 4 images/core. Per core an
implicit-GEMM conv: C_in=128 is the partition (contraction) dim; for each
(kh, kw) tap a [128ci x 128co] weight tile multiplies a shifted window of the
column-padded input image held in SBUF, accumulating into PSUM over the 9 taps.
Output rows are processed in chunks of 8 (free dim 8*56=448 <= 512 PSUM bank).
Matmuls run in float16 (fp16 keeps ~2.6e-4 rel err) with fp32 PSUM accumulate.

Layout details (all tuned from perfetto traces):
- Two SBUF copies of each input slice: copy A with the row interior at column
  1 (serves kw=0 and kw=2 taps) and copy B at column 2 (serves kw=1). This
  keeps every matmul rhs at an even fp16 element offset; odd offsets cost ~18
  extra PE cycles per matmul (SBUF word-split reads).
- No zero pad ROWS: boundary chunks instead shrink the kh taps that would
  read them (448 -> 392 free, PSUM sub-range), saving ~1.1us of PE work.
  Copy A keeps zero pad COLUMNS (elems 0 and 57 of each 58-elem row).
- PE warmup: dummy matmuls opened ASAP (wu memset on the gpsimd queue, which
  starts ~1us before vector) keep the PE busy while head DMAs land, so the
  HAM clock gate (opens ~6us after first PE activity, half clock until then)
  is already open when the real stream runs.
- Head DMAs: first x slice is only 9 rows (enough for chunk 0) and the tap-
  first weight block is a separate small DMA, so the first real matmul's
  dependencies land as early as possible.
- Outputs: full-half bulk DMAs for images 0..2 (12.5KB/partition
  descriptors), but CHUNK-wise for the whole last image so the final chunk's
  writeout is not queued behind a 1.6MB bulk transfer in the DMA FIFOs.
"""
import numpy as np
from contextlib import ExitStack

N_FULL, C_IN, H, W = 32, 128, 56, 56
C_OUT, KS = 256, 3
N_CORES = 8
N_PER = N_FULL // N_CORES          # 4 images per core
PIX = H * W                         # 3136
ROWS = 8                            # output rows per psum chunk
RC = H // ROWS                      # 7 chunks
NF = ROWS * W                       # 448 free elems per matmul
NARROW = (ROWS - 1) * W             # 392 free elems for boundary taps
SW = 58                             # padded row stride (56 + 2 border cols)

T_ROWS = 33                         # top tile: x rows 0..32 (chunks 0-3)
B_ROWS = 25                         # bottom tile: x rows 31..55 (chunks 4-6)
B0 = 31                             # first x row held in the bottom tile
XT_A = 9                            # first sub-DMA: x rows 0..8 (chunk 0)
XT_B = 12                           # second: x rows 9..20
XT_C = T_ROWS - XT_A - XT_B         # third: x rows 21..32

# tap order per chunk kind: (kh, kw) tuples. start=first, stop=last.
# narrow taps (boundary rows) must not be first or last.
TAPS_MID = [(0, 0), (0, 1), (0, 2), (1, 0), (1, 1), (1, 2), (2, 0), (2, 1), (2, 2)]
# top chunk leads with the kw=1 taps: for image 0 they run straight off the
# raw (unpadded) first DMA slice -- the center column needs no padding and
# raw rows are contiguous -- so real work starts before any pad copy lands
TAPS_TOP = [(1, 1), (2, 1), (0, 1), (1, 0), (1, 2), (0, 0), (0, 2), (2, 0), (2, 2)]
TAPS_BOT = [(0, 0), (0, 1), (0, 2), (2, 0), (2, 1), (2, 2), (1, 0), (1, 1), (1, 2)]

_CACHE = {}


def _build():
    import concourse.tile as tile
    from concourse import mybir, bacc

    f32 = mybir.dt.float32
    f16 = mybir.dt.float16

    nc = bacc.Bacc("TRN2", target_bir_lowering=False, debug=False)
    x_d = nc.dram_tensor("x", [N_PER, C_IN, PIX], f16, kind="ExternalInput").ap()
    # host-pretransposed: [ci, half, k, co_half] (half-major, contiguous per
    # half); within each half tap k=3 (the first tap chunk 0 runs) is stored
    # first: host order TAPS_TOP.
    w_d = nc.dram_tensor("w", [C_IN, 2, KS * KS, 128], f16, kind="ExternalInput").ap()
    b_d = nc.dram_tensor("b", [C_OUT], f32, kind="ExternalInput").ap()
    # fp16 output (upcast on host): halves output DMA bytes; ~5e-4 rel err
    y_d = nc.dram_tensor("y", [N_PER, C_OUT, PIX], f16, kind="ExternalOutput").ap()

    # host-order index of tap (kh, kw) inside a weight half (TAPS_TOP order)
    widx = {t: i for i, t in enumerate(TAPS_TOP)}

    with tile.TileContext(nc) as tc:
        with ExitStack() as ctx:
            wp = ctx.enter_context(tc.tile_pool(name="wp", bufs=1))
            xrawta = ctx.enter_context(tc.tile_pool(name="xrawta", bufs=1))
            xrawtb = ctx.enter_context(tc.tile_pool(name="xrawtb", bufs=1))
            xrawtc = ctx.enter_context(tc.tile_pool(name="xrawtc", bufs=1))
            xrawb = ctx.enter_context(tc.tile_pool(name="xrawb", bufs=1))
            # images 1-3 land whole, prefetched at kernel start with no deps:
            # their descriptors sit behind image-0's in the ring FIFOs, and all
            # input traffic completes before the first bulk output DMA can
            # starve it (big output descriptors occupy a queue ~0.5us each).
            xrawf = ctx.enter_context(tc.tile_pool(name="xrawf", bufs=N_PER - 1))
            xpadt = ctx.enter_context(tc.tile_pool(name="xpadt", bufs=4))
            xpadb = ctx.enter_context(tc.tile_pool(name="xpadb", bufs=4))
            pp = ctx.enter_context(tc.tile_pool(name="pp", bufs=6, space="PSUM"))
            op = ctx.enter_context(tc.tile_pool(name="op", bufs=2))

            # PE warmup: the clock ramps to full speed only after ~3-6us of
            # CONTINUOUS PE execution, and re-gates after ~1us idle. Keep the
            # PE busy with dummies from as early as possible until real work
            # is ready (~8.5us). The first dummies read the framework's
            # const-AP (materialized during init, no runtime memset dep) so
            # they start right at the post-init barrier on every core; then
            # wu-based dummies, shrinking toward the end for a fine handoff.
            c0 = nc.const_aps.tensor(0.0, [128, 1], f32)
            wups = pp.tile([128, NF], f32, tag="ps")
            for _ in range(10):
                nc.tensor.matmul(wups[0:1, 0:1], c0, c0, start=True, stop=True)
            wu = wp.tile([128, 224], f16)
            nc.gpsimd.memset(wu[:], 0.0)
            # overprovision the runway: a core whose inputs land late pays
            # ~1.5us for a PE idle (gap + clock re-gate), while an excess
            # dummy costs only ~45ns of displaced half-clock work
            for _ in range(7):
                nc.tensor.matmul(wups[:, 0:224], wu[:, 0:128], wu[:], start=True, stop=True)
            for _ in range(8):
                nc.tensor.matmul(wups[:, 0:112], wu[:, 0:128], wu[:, 0:112], start=True, stop=True)

            # Weight half 0 in two pieces on the ACT ring: chunk 0's first six
            # taps' weights (192KB) gate the first matmuls; the rest follows.
            w_r = wp.tile([C_IN, 2 * KS * KS * 128], f16)
            w_r4 = w_r[:].rearrange("p (h k co) -> p h k co", h=2, k=KS * KS)
            nc.scalar.dma_start(
                w_r4[:, 0, 0:6], w_d[:, 0, 0:6].rearrange("ci k co -> ci (k co)")
            )
            nc.scalar.dma_start(
                w_r4[:, 0, 6:9], w_d[:, 0, 6:9].rearrange("ci k co -> ci (k co)")
            )

            bias_sb = wp.tile([128, 2], f32)

            for n in range(N_PER):
                if n == 0:
                    # image 0: top interior in three slices so chunk 0
                    # unblocks as early as possible
                    xrta = xrawta.tile([C_IN, XT_A * W], f16)
                    nc.sync.dma_start(xrta[:], x_d[n, :, 0 : XT_A * W])
                    xrtb = xrawtb.tile([C_IN, XT_B * W], f16)
                    nc.sync.dma_start(xrtb[:], x_d[n, :, XT_A * W : (XT_A + XT_B) * W])
                    xrtc = xrawtc.tile([C_IN, XT_C * W], f16)
                    nc.sync.dma_start(xrtc[:], x_d[n, :, (XT_A + XT_B) * W : T_ROWS * W])
                    xrb = xrawb.tile([C_IN, B_ROWS * W], f16)
                    nc.sync.dma_start(xrb[:], x_d[n, :, B0 * W : (B0 + B_ROWS) * W])
                    top_srcs = [(xrta[:], 0, XT_A), (xrtb[:], XT_A, XT_A + XT_B),
                                (xrtc[:], XT_A + XT_B, T_ROWS)]
                    bot_src = xrb[:]
                    # weight half 1 + bias queue behind the critical pieces
                    nc.scalar.dma_start(
                        w_r4[:, 1], w_d[:, 1].rearrange("ci k co -> ci (k co)")
                    )
                    nc.scalar.dma_start(bias_sb[:], b_d.rearrange("(h p) -> p h", h=2))
                else:
                    # images 1-3: one whole-image DMA, prefetched (no deps)
                    xrf = xrawf.tile([C_IN, PIX], f16)
                    nc.sync.dma_start(xrf[:], x_d[n])
                    top_srcs = [(xrf[:, 0 : T_ROWS * W], 0, T_ROWS)]
                    bot_src = xrf[:, B0 * W : (B0 + B_ROWS) * W]

                # copy A: interior at col 1 (kw=0,2 taps; even rhs offsets
                # 0/2); zero border cols 0 and 57.  copy B: interior at col 2
                # (kw=1 taps; even rhs offset 2), no borders needed.
                xptA = xpadt.tile([C_IN, T_ROWS * SW], f16)
                xptA3 = xptA[:].rearrange("p (a b) -> p a b", a=T_ROWS)
                xptB = xpadt.tile([C_IN, T_ROWS * SW], f16)
                xptB3 = xptB[:].rearrange("p (a b) -> p a b", a=T_ROWS)
                nc.vector.memset(xptA3[:, :, 0:1], 0.0)
                nc.vector.memset(xptA3[:, :, SW - 1 : SW], 0.0)
                # all copies on the DVE (Pool runs them ~6.5x slower); the
                # bias-adds live on the ACT engine instead, so a long copy
                # can never head-of-line-block an ADD (which would back up
                # PSUM and stall the PE at image boundaries).
                # interleave A/B per slice: chunk 0 needs both A and B of the
                # first slice ASAP
                for (src, a0, a1) in top_srcs:
                    src3 = src.rearrange("p (a b) -> p a b", a=a1 - a0)
                    nc.vector.tensor_copy(xptA3[:, a0:a1, 1 : 1 + W], src3)
                    if n == 0:
                        # images 1-3 run kw=1 taps off the raw tile directly;
                        # only image 0 (split raw pieces) needs the B copy
                        nc.vector.tensor_copy(xptB3[:, a0:a1, 2 : 2 + W], src3)

                xpbA = xpadb.tile([C_IN, B_ROWS * SW], f16)
                xpbA3 = xpbA[:].rearrange("p (a b) -> p a b", a=B_ROWS)
                xpbB = xpadb.tile([C_IN, B_ROWS * SW], f16)
                xpbB3 = xpbB[:].rearrange("p (a b) -> p a b", a=B_ROWS)
                nc.vector.memset(xpbA3[:, :, 0:1], 0.0)
                nc.vector.memset(xpbA3[:, :, SW - 1 : SW], 0.0)
                xrb3 = bot_src.rearrange("p (a b) -> p a b", a=B_ROWS)
                nc.vector.tensor_copy(xpbA3[:, :, 1 : 1 + W], xrb3)
                if n == 0:
                    nc.vector.tensor_copy(xpbB3[:, :, 2 : 2 + W], xrb3)

                out_sb = op.tile([128, 2 * PIX], f16)
                last_img = n == N_PER - 1
                for half in range(2):
                    # the very last half ends with 6-row + 2-row chunks so the
                    # post-final-matmul ADD+DMA tail is tiny
                    if last_img and half == 1:
                        chunks = [(i * ROWS, ROWS) for i in range(6)] + [(48, 6), (54, 2)]
                    else:
                        chunks = [(i * ROWS, ROWS) for i in range(RC)]
                    for ci, (r0c, nr) in enumerate(chunks):
                        nfc = nr * W
                        ps = pp.tile([128, NF], f32)
                        top_chunk = r0c == 0
                        bot_chunk = r0c + nr == H
                        taps = TAPS_TOP if top_chunk else (TAPS_BOT if bot_chunk else TAPS_MID)
                        for i, (kh, kw) in enumerate(taps):
                            lhsT = w_r4[:, half, widx[(kh, kw)], :]
                            # output rows r0c..r0c+nr-1 read x rows r0c+kh-1..
                            r0 = r0c + kh - 1
                            narrow_top = top_chunk and kh == 0     # skip out row 0
                            narrow_bot = bot_chunk and kh == 2     # skip out row 55
                            if r0c < 32:
                                A3, B3, base = xptA3, xptB3, 0
                            else:
                                A3, B3, base = xpbA3, xpbB3, B0
                            src3 = B3 if kw == 1 else A3
                            coff = 2 if kw == 1 else kw
                            if n == 0 and top_chunk and kw == 1:
                                # image 0's first chunk: kw=1 taps straight
                                # off the raw DMA slice (no pad copy dep)
                                src3 = xrta[:].rearrange("p (a b) -> p a b", a=XT_A)
                                coff = 0
                            if kw == 1 and n > 0:
                                # kw=1 needs no column padding: read the raw
                                # whole-image tile as ONE contiguous free run.
                                # A row-structured rhs AP costs ~1 PE cycle
                                # per row boundary; the flat run costs none.
                                nrows = nr - 1 if (narrow_top or narrow_bot) else nr
                                rr0 = 0 if narrow_top else r0
                                rhs = xrf[:, rr0 * W : (rr0 + nrows) * W]
                                if narrow_top:
                                    dst = ps[:, W:nfc]
                                elif narrow_bot:
                                    dst = ps[:, 0 : nfc - W]
                                else:
                                    dst = ps[:, 0:nfc]
                            elif narrow_top:
                                rhs = src3[:, 0 : nr - 1, coff : coff + W]
                                dst = ps[:, W:nfc]
                            elif narrow_bot:
                                lr = r0 - base
                                rhs = src3[:, lr : lr + nr - 1, coff : coff + W]
                                dst = ps[:, 0 : nfc - W]
                            else:
                                lr = r0 - base
                                rhs = src3[:, lr : lr + nr, coff : coff + W]
                                dst = ps[:, 0:nfc]
                            nc.tensor.matmul(
                                dst, lhsT, rhs,
                                start=(i == 0), stop=(i == KS * KS - 1),
                            )
                        # psum -> sbuf with per-channel bias add (f32 -> f16)
                        # on the ACT engine: keeps the DVE free for pad copies
                        lo = half * PIX + r0c * W
                        nc.scalar.activation(
                            out_sb[:, lo : lo + nfc],
                            ps[:, 0:nfc],
                            mybir.ActivationFunctionType.Identity,
                            bias=bias_sb[:, half : half + 1],
                        )
                        if last_img:
                            # last image: stream output out as produced so
                            # nothing bulky queues ahead of the tail. Pair up
                            # 8-row chunks (fewer DMAs -> fewer semaphores to
                            # clear in teardown); alternate rings so
                            # consecutive writes drain in parallel.
                            paired = (
                                nr == ROWS and ci % 2 == 0
                                and ci + 1 < len(chunks) and chunks[ci + 1][1] == ROWS
                            )
                            if paired:
                                pass  # written together with the next chunk
                            else:
                                if nr == 6:
                                    continue  # written merged with the final mini chunk
                                w0 = (r0c - ROWS) * W if (nr == ROWS and ci % 2 == 1) else r0c * W
                                if nr == 2:
                                    # final mini chunk: one merged 8-row write
                                    # (rows 48-55; bigger descriptors, single
                                    # issue op) on the scalar queue right
                                    # behind its own bias-add -- the in-order
                                    # queue guarantees the 6-row chunk's add
                                    # already ran, and no cross-engine
                                    # semaphore hop on the tail-critical DMA
                                    w0 = (r0c - 6) * W
                                    eng = nc.scalar
                                else:
                                    eng = nc.sync if (half == 0) == (ci % 2 == 0) else nc.gpsimd
                                eng.dma_start(
                                    y_d[n, half * 128 : (half + 1) * 128, w0 : r0c * W + nfc],
                                    out_sb[:, half * PIX + w0 : lo + nfc],
                                )
                    if not last_img:
                        eng = nc.scalar if half == 0 else nc.gpsimd
                        eng.dma_start(
                            y_d[n, half * 128 : (half + 1) * 128, :],
                            out_sb[:, half * PIX : (half + 1) * PIX],
                        )
    nc.compile()
    return nc


def _get_nc():
    if "nc" not in _CACHE:
        _CACHE["nc"] = _build()
    return _CACHE["nc"]


def _prep_inputs(x, weight, bias):
    # fp16 on host: halves input DMA bytes and drops the on-device casts
    x = np.ascontiguousarray(
        np.asarray(x, dtype=np.float32).astype(np.float16).reshape(N_FULL, C_IN, PIX)
    )
    # [co, ci, kh, kw] -> [ci, half, k, co_half], half-major; taps within a
    # half stored in TAPS_TOP order so chunk-0's first weights DMA first.
    w4 = (
        np.transpose(np.asarray(weight, dtype=np.float32), (1, 2, 3, 0))
        .reshape(C_IN, KS * KS, 2, 128)
        .transpose(0, 2, 1, 3)
    )  # [ci, half, k(row-major), co]
    perm = [kh * KS + kw for (kh, kw) in TAPS_TOP]
    w_t = np.ascontiguousarray(w4[:, :, perm, :].astype(np.float16))
    b = np.ascontiguousarray(bias, dtype=np.float32)
    return x, w_t, b


def kernel(x, weight, bias):
    from concourse.bass_utils import run_bass_kernel_spmd

    x, w_t, b = _prep_inputs(x, weight, bias)
    nc = _get_nc()
    in_maps = [
        {"x": x[i * N_PER : (i + 1) * N_PER], "w": w_t, "b": b}
        for i in range(N_CORES)
    ]
    res = run_bass_kernel_spmd(nc, in_maps, list(range(N_CORES)))
    y = np.concatenate(
        [
            res.results[i]["y"].reshape(N_PER, C_OUT, H, W).astype(np.float32)
            for i in range(N_CORES)
        ],
        axis=0,
    )
    return y
